# revision 13
# baseline (speedup 1.0000x reference)
"""ClusterLoss Bass/Tile kernel for Trainium2 (8 NeuronCores, data parallel).

Strategy (v2: TensorE dot products, ragged DMA)
-----------------------------------------------
Samples are globally sorted by hn_count (descending) and stripe-dealt to the
8 cores, so each core's in-core rank k holds a sample whose negative count is
nearly identical across cores.  Pairs of ranks share a compile-time negative
bound N_g, making per-pair DMA nearly exactly the valid (ragged) data.

All dot products (q.hn, q.fn, q.k, q.k2) run on the Tensor engine: for each
sample, a [128, 32] weight tile holding the sample's qT d-chunk in column
(rank%32) and zeros elsewhere is multiplied against the raw transposed
hn/fn/k/k2 columns ([d', W] bf16, streamed at 1 col/cycle), accumulating into
psum rows [32j, 32j+32) via column tiling.  After 128 samples, the psum bank
holds lneg[sample, n] plus fn dots and the k/k2 dots; ScalarE copies it out
with the 1/TEMP scale folded in.  The DVE only builds the tiny q-column
weights and runs the masked-logsumexp / cross-entropy / BML epilogue, so the
kernel is DMA-bound rather than vector-bound.

Per-block psum layout (columns): [0,32) fn dots | 32 q.k | 33 q.k2 |
[34, 34+N) hn dots.  Rows = samples.  The psum bank is zeroed by the DVE
once per block and every matmul accumulates (start=False).

The program is JIT-specialized to the per-pair bounds tuple derived from the
counts at call time and cached, so repeated calls with the same raggedness
profile reuse the compiled NEFF.
"""

from contextlib import ExitStack

import numpy as np

import concourse.bass as bass
import concourse.bacc as bacc
import concourse.tile as tile
from concourse import mybir
from concourse import bass_utils

N_CORES = 8
B, D, N_MAX, M_MAX = 2048, 512, 256, 32
B_LOC = B // N_CORES          # 256 samples per core
PBLK = 128                    # partition block (samples per psum bank)
PRE = M_MAX + 2               # fn cols + k + k2 before the hn columns
GRP = 8                       # samples per DMA group (uniform W within group)
NGRP = B_LOC // GRP           # 32 groups per core
HBUFS = 7                     # group-tile ring

TEMP, ALPHA, BETA, LAMBDA_BML = 0.07, 0.4, 0.2, 0.2
NEG = -1e30
EXP_CLAMP = -87.0

F32 = mybir.dt.float32
F16 = mybir.dt.float16
BF16 = mybir.dt.bfloat16
I32 = mybir.dt.int32
AF = mybir.ActivationFunctionType
OP = mybir.AluOpType
AX = mybir.AxisListType


def _col_ap(qw, r):
    """Column r of a [128, 4, 32] tile: 4 elements with stride 32."""
    a = qw[:]
    return bass.AP(tensor=a.tensor, offset=a.offset + r, ap=[a.ap[0], [32, 4]])


def _emit(tc, Ns, hp, qT, hc, fc, out):
    nc = tc.nc
    NB = (max(Ns[0], 1), max(Ns[NGRP // 2], 1))
    with ExitStack() as ctx:
        hpool = ctx.enter_context(tc.tile_pool(name="hp", bufs=HBUFS))
        qpool = ctx.enter_context(tc.tile_pool(name="qp", bufs=1))
        mpool = ctx.enter_context(tc.tile_pool(name="med", bufs=2))
        smpool = ctx.enter_context(tc.tile_pool(name="sm", bufs=2))
        cpool = ctx.enter_context(tc.tile_pool(name="cst", bufs=1))
        ppool = ctx.enter_context(tc.tile_pool(name="ps", bufs=2, space="PSUM"))
        fpool = ctx.enter_context(tc.tile_pool(name="fin", bufs=1, space="PSUM"))

        # constants
        iota_i = cpool.tile([PBLK, N_MAX], I32, tag="iota_i", name="iota_i")
        nc.gpsimd.iota(iota_i[:], pattern=[[1, N_MAX]], base=0, channel_multiplier=0)
        iota_f = cpool.tile([PBLK, N_MAX], F32, tag="iota_f", name="iota_f")
        nc.vector.tensor_copy(out=iota_f[:], in_=iota_i[:])
        ones = cpool.tile([PBLK, 1], F32, tag="ones", name="ones")
        nc.vector.memset(ones[:], 1.0)
        alpha_t = cpool.tile([PBLK, 1], F32, tag="alpha_t", name="alpha_t")
        nc.vector.memset(alpha_t[:], ALPHA)
        nbeta_t = cpool.tile([PBLK, 1], F32, tag="nbeta_t", name="nbeta_t")
        nc.vector.memset(nbeta_t[:], -BETA)

        qT_t = qpool.tile([PBLK, B_LOC, 4], BF16, tag="qT", name="qT_t")
        nc.sync.dma_start(out=qT_t[:], in_=qT[:])

        # q-column weight tiles: per (block, strip, parity), zeroed once
        qw = {}
        for b in range(2):
            for j in range(4):
                for par in range(2):
                    t = cpool.tile([PBLK, 4, 32], BF16, tag=f"qw{b}{j}{par}",
                                   name=f"qw{b}{j}{par}")
                    nc.vector.memset(t[:], 0.0)
                    qw[(b, j, par)] = t

        def sm(tagname, dt=F32, w=1):
            return smpool.tile([PBLK, w], dt, tag=tagname, name=tagname)

        hp_ap = hp[:]
        off = 0
        blk_contribs = []
        for b in range(2):
            ps = ppool.tile([PBLK, 512], F32, tag="ps", name=f"ps{b}")
            nc.vector.memset(ps[:], 0.0)

            hc_i = sm("hc_i", I32)
            nc.sync.dma_start(out=hc_i[:], in_=hc[b])
            fc_i = sm("fc_i", I32)
            nc.sync.dma_start(out=fc_i[:], in_=fc[b])
            hc_f = sm("hc_f")
            nc.vector.tensor_copy(out=hc_f[:], in_=hc_i[:])
            fc_f = sm("fc_f")
            nc.vector.tensor_copy(out=fc_f[:], in_=fc_i[:])

            for gi in range(NGRP // 2):
                g = (NGRP // 2) * b + gi
                N = Ns[g]
                W = PRE + N
                t = hpool.tile([PBLK, GRP, 4, W], BF16, tag="hpt", name=f"hp{g}")
                src = bass.AP(
                    tensor=hp_ap.tensor,
                    offset=off,
                    ap=[[4 * W, PBLK], [PBLK * 4 * W, GRP], [W, 4], [1, W]],
                )
                trig = (nc.sync, nc.gpsimd, nc.scalar)[g % 3]
                trig.dma_start(out=t[:], in_=src)
                for i in range(GRP):
                    sb = GRP * gi + i        # in-block rank
                    s = PBLK * b + sb        # in-core rank
                    j, r = sb // 32, sb % 32
                    w_t = qw[(b, j, sb % 2)]
                    if r >= 2:
                        nc.vector.memset(_col_ap(w_t, r - 2), 0.0)
                    nc.vector.tensor_copy(out=_col_ap(w_t, r),
                                          in_=qT_t[:, s, :])
                    last = (gi == NGRP // 2 - 1 and i == GRP - 1)
                    for dc in range(4):
                        nc.tensor.matmul(
                            ps[32 * j:32 * j + 32, 0:W],
                            lhsT=w_t[:, dc, :],
                            rhs=t[:, i, dc, :],
                            start=False, stop=(last and dc == 3),
                            tile_position=(0, 32 * j),
                            skip_group_check=True,
                        )
                off += GRP * 4 * PBLK * W

            # ---- extraction ----
            NBb = NB[b]
            lneg = mpool.tile([PBLK, NBb], F32, tag="lneg", name="lneg")
            nc.scalar.activation(out=lneg[:], in_=ps[:, PRE:PRE + NBb],
                                 func=AF.Copy, scale=1.0 / TEMP)
            fnd = smpool.tile([PBLK, M_MAX], F32, tag="fnd", name="fnd")
            nc.scalar.activation(out=fnd[:], in_=ps[:, 0:M_MAX],
                                 func=AF.Copy, scale=1.0)
            lpos = sm("lpos")
            nc.scalar.activation(out=lpos[:], in_=ps[:, 32:33],
                                 func=AF.Copy, scale=1.0 / TEMP)
            lposnb = sm("lposnb")
            nc.scalar.activation(out=lposnb[:], in_=ps[:, 33:34],
                                 func=AF.Copy, scale=1.0 / TEMP)
            simpos = sm("simpos")
            nc.vector.tensor_scalar_mul(out=simpos[:], in0=lpos[:],
                                        scalar1=TEMP)

            # ---- masked logsumexp over negatives ----
            mneg = mpool.tile([PBLK, NBb], F32, tag="mneg", name="mneg")
            nc.vector.tensor_scalar(
                out=mneg[:], in0=iota_f[:, :NBb], scalar1=hc_f[:],
                scalar2=NEG, op0=OP.is_ge, op1=OP.mult,
            )
            nc.vector.tensor_add(out=lneg[:], in0=lneg[:], in1=mneg[:])
            mrow = sm("mrow")
            nc.vector.tensor_reduce(out=mrow[:], in_=lneg[:], axis=AX.X,
                                    op=OP.max)
            nmrow = sm("nmrow")
            nc.vector.tensor_scalar_mul(out=nmrow[:], in0=mrow[:], scalar1=-1.0)
            expin = mpool.tile([PBLK, NBb], F32, tag="expin", name="expin")
            nc.vector.tensor_scalar(
                out=expin[:], in0=lneg[:], scalar1=nmrow[:],
                scalar2=EXP_CLAMP, op0=OP.add, op1=OP.max,
            )
            expout = mpool.tile([PBLK, NBb], F32, tag="expout", name="expout")
            sumexp = sm("sumexp")
            nc.scalar.activation(
                out=expout[:], in_=expin[:], func=AF.Exp,
                accum_out=sumexp[:],
            )
            lse = sm("lse")
            nc.scalar.activation(out=lse[:], in_=sumexp[:], func=AF.Ln)
            nc.vector.tensor_add(out=lse[:], in0=lse[:], in1=mrow[:])

            # ce(lp) = logaddexp(lp, lse) - lp
            def ce(lp, tag):
                mm = sm("mm" + tag)
                nc.vector.tensor_max(out=mm[:], in0=lp[:], in1=lse[:])
                nmm = sm("nmm" + tag)
                nc.vector.tensor_scalar_mul(out=nmm[:], in0=mm[:], scalar1=-1.0)
                e1 = sm("e1" + tag)
                nc.vector.tensor_scalar(
                    out=e1[:], in0=lp[:], scalar1=nmm[:], scalar2=EXP_CLAMP,
                    op0=OP.add, op1=OP.max,
                )
                nc.scalar.activation(out=e1[:], in_=e1[:], func=AF.Exp)
                e2 = sm("e2" + tag)
                nc.vector.tensor_scalar(
                    out=e2[:], in0=lse[:], scalar1=nmm[:], scalar2=EXP_CLAMP,
                    op0=OP.add, op1=OP.max,
                )
                nc.scalar.activation(out=e2[:], in_=e2[:], func=AF.Exp)
                s12 = sm("s12" + tag)
                nc.vector.tensor_add(out=s12[:], in0=e1[:], in1=e2[:])
                nc.scalar.activation(out=s12[:], in_=s12[:], func=AF.Ln)
                cev = sm("ce" + tag)
                nc.vector.tensor_add(out=cev[:], in0=s12[:], in1=mm[:])
                nc.vector.tensor_sub(out=cev[:], in0=cev[:], in1=lp[:])
                return cev

            cep = ce(lpos, "p")
            cenb = ce(lposnb, "n")

            # ---- BML term ----
            maskf = sm("maskf", w=M_MAX)
            nc.vector.tensor_scalar(
                out=maskf[:], in0=iota_f[:, :M_MAX], scalar1=fc_f[:],
                scalar2=None, op0=OP.is_lt,
            )
            nc.vector.tensor_mul(out=fnd[:], in0=fnd[:], in1=maskf[:])
            sfn = sm("sfn")
            nc.vector.tensor_reduce(out=sfn[:], in_=fnd[:], axis=AX.X, op=OP.add)
            den = sm("den")
            nc.vector.tensor_scalar_max(out=den[:], in0=fc_f[:], scalar1=1.0)
            rden = sm("rden")
            nc.vector.reciprocal(out=rden[:], in_=den[:])
            simfn = sm("simfn")
            nc.vector.tensor_mul(out=simfn[:], in0=sfn[:], in1=rden[:])
            delta = sm("delta")
            nc.vector.tensor_sub(out=delta[:], in0=simfn[:], in1=simpos[:])
            r1 = sm("r1")
            nc.scalar.activation(out=r1[:], in_=delta[:], func=AF.Relu,
                                 bias=alpha_t[:], scale=1.0)
            r2 = sm("r2")
            nc.scalar.activation(out=r2[:], in_=delta[:], func=AF.Relu,
                                 bias=nbeta_t[:], scale=-1.0)
            bml = sm("bml")
            nc.vector.tensor_add(out=bml[:], in0=r1[:], in1=r2[:])

            vh = sm("vh")
            nc.vector.tensor_scalar(out=vh[:], in0=hc_f[:], scalar1=0.0,
                                    scalar2=None, op0=OP.is_gt)
            vf = sm("vf")
            nc.vector.tensor_scalar(out=vf[:], in0=fc_f[:], scalar1=0.0,
                                    scalar2=None, op0=OP.is_gt)
            vb = sm("vb")
            nc.vector.tensor_mul(out=vb[:], in0=vh[:], in1=vf[:])

            contrib = smpool.tile([PBLK, 5], F32, tag="contrib", name="contrib")
            nc.vector.tensor_mul(out=contrib[:, 0:1], in0=cep[:], in1=vh[:])
            nc.vector.tensor_mul(out=contrib[:, 1:2], in0=cenb[:], in1=vh[:])
            nc.vector.tensor_mul(out=contrib[:, 2:3], in0=bml[:], in1=vb[:])
            nc.vector.tensor_copy(out=contrib[:, 3:4], in_=vh[:])
            nc.vector.tensor_copy(out=contrib[:, 4:5], in_=vb[:])
            blk_contribs.append(contrib)

        tot = blk_contribs[0]
        nc.vector.tensor_add(out=tot[:], in0=tot[:], in1=blk_contribs[1][:])

        fres = fpool.tile([5, 1], F32, tag="ps5", name="ps5")
        nc.tensor.matmul(fres[:], lhsT=tot[:], rhs=ones[:], start=True, stop=True)
        res = smpool.tile([5, 1], F32, tag="res", name="res")
        nc.scalar.copy(out=res[:], in_=fres[:])
        nc.sync.dma_start(out=out[:], in_=res[:])


def _build(Ns):
    nc = bacc.Bacc("TRN2", target_bir_lowering=False, debug=False)
    total = sum(GRP * 512 * (PRE + N) for N in Ns)
    hp = nc.dram_tensor("hp", [total], BF16, kind="ExternalInput")
    qT = nc.dram_tensor("qT", [PBLK, B_LOC, 4], BF16, kind="ExternalInput")
    hc = nc.dram_tensor("hn_counts", [2, PBLK, 1], I32, kind="ExternalInput")
    fc = nc.dram_tensor("fn_counts", [2, PBLK, 1], I32, kind="ExternalInput")
    out = nc.dram_tensor("out", [5, 1], F32, kind="ExternalOutput")
    with tile.TileContext(nc) as tc:
        _emit(tc, Ns, hp, qT, hc, fc, out)
    nc.compile()
    return nc


_NC_CACHE = {}


def _get_nc(key):
    if key not in _NC_CACHE:
        _NC_CACHE[key] = _build(key)
    return _NC_CACHE[key]


def plan(hn_counts):
    """Count-sorted stripe schedule: rank k of core c = order[8k + c].
    Group g (ranks GRP*g .. GRP*g+GRP-1) gets negative bound Ns[g] = max
    count in the group across all cores = sorted_count[8*GRP*g]."""
    counts = np.asarray(hn_counts)
    order = np.argsort(-counts, kind="stable")
    cs = counts[order]
    Ns = tuple(int(cs[N_CORES * GRP * g]) for g in range(NGRP))
    return order, Ns


def make_in_maps(q, k, k2, hn, fn, hn_counts, fn_counts):
    import ml_dtypes
    bf16 = ml_dtypes.bfloat16
    q = np.asarray(q, np.float32)
    k = np.asarray(k, np.float32)
    k2 = np.asarray(k2, np.float32)
    hn = np.asarray(hn, np.float32)
    fn = np.asarray(fn, np.float32)
    hn_counts = np.asarray(hn_counts, np.int32)
    fn_counts = np.asarray(fn_counts, np.int32)

    order, Ns = plan(hn_counts)
    ranks = order.reshape(B_LOC, N_CORES)    # [rank, core]
    sizes = [GRP * 512 * (PRE + N) for N in Ns]
    offs = np.concatenate([[0], np.cumsum(sizes)])
    total = int(offs[-1])

    # pre-transposed per-sample views
    fnT = fn.reshape(B, M_MAX, 4, 128)
    kT = k.reshape(B, 4, 128)
    k2T = k2.reshape(B, 4, 128)
    hnT = hn.reshape(B, N_MAX, 4, 128)

    in_maps = []
    for c in range(N_CORES):
        sc = ranks[:, c]
        hp = np.empty(total, bf16)
        for g in range(NGRP):
            N = Ns[g]
            W = PRE + N
            base = int(offs[g])
            for i in range(GRP):
                s = int(sc[GRP * g + i])
                tmp = np.empty((128, 4, W), bf16)
                tmp[:, :, :M_MAX] = fnT[s].transpose(2, 1, 0)
                tmp[:, :, M_MAX] = kT[s].T
                tmp[:, :, M_MAX + 1] = k2T[s].T
                if N:
                    tmp[:, :, PRE:] = hnT[s, :N].transpose(2, 1, 0)
                hp[base + i * 512 * W: base + (i + 1) * 512 * W] = tmp.ravel()
        qTc = np.ascontiguousarray(
            q[sc].reshape(B_LOC, 4, 128).transpose(2, 0, 1)).astype(bf16)
        in_maps.append({
            "hp": hp,
            "qT": qTc,
            "hn_counts": hn_counts[sc].reshape(2, PBLK, 1),
            "fn_counts": fn_counts[sc].reshape(2, PBLK, 1),
        })
    return in_maps, Ns


def combine_partials(results):
    parts = np.stack([np.asarray(r["out"], np.float64).reshape(5) for r in results])
    cl_s, clnb_s, bml_s, nv, nb = parts.sum(axis=0)
    n_valid = max(nv, 1.0)
    cl = cl_s / n_valid
    clnb = clnb_s / n_valid
    bml_mean = (bml_s / nb) if nb > 0 else 0.0
    lbml = LAMBDA_BML * bml_mean
    tot = cl + clnb + lbml
    return np.array([tot, cl, lbml, clnb], np.float32)


def run_spmd(in_maps, bounds, **kwargs):
    nc = _get_nc(tuple(bounds))
    return bass_utils.run_bass_kernel_spmd(
        nc, in_maps, core_ids=list(range(N_CORES)), **kwargs
    )


def kernel(q, k, k2, hn, fn, hn_counts, fn_counts):
    in_maps, Ns = make_in_maps(q, k, k2, hn, fn, hn_counts, fn_counts)
    res = run_spmd(in_maps, Ns)
    return combine_partials(res.results)
